# revision 1
# baseline (speedup 1.0000x reference)
"""Self-contained Trainium2 kernel for nn_Attention_56607668961538 (v2).

kernel(**inputs) takes the FULL unsharded inputs (B=16, N=1024, C=1024),
shards data-parallel over batch across 8 NeuronCores (B_local=2 each),
runs a Bass/Tile attention kernel per core via run_bass_kernel_spmd, and
gathers the full output.

v2 strategy vs v1:
  - all intermediates (Q/K features, V in token-major layout, attention
    output) stay resident in SBUF; no DRAM roundtrips.
  - bf16 storage + matmul operands everywhere precision allows (HW runs
    bf16 and f32r matmuls at the same rate, but bf16 halves DMA + SBUF).
  - rmsnorm sum-of-squares accumulated in PSUM (mask matmuls with
    start/stop groups) instead of DVE adds.
  - one deferred ACT Sqrt per batch + DVE reciprocal_approx_fast for the
    rms/softmax normalizers (2 ACT table loads per batch, the minimum).
  - both q and k pre-scaled by their rms normalizers (selector-matmul
    broadcast + in-place DVE mul), so the attention exp is scale-free.
  - weights loaded once per rep (resident across both batch elements);
    lead-in phase1 and tail projection borrow the idle attention PSUM
    pools; emission order interleaves batches so phase1(b+1) overlaps
    attention(b) and proj(b) overlaps attention(b+1).
"""

import sys

sys.path.insert(0, "/opt/trn_rl_repo")

import numpy as np

from contextlib import ExitStack

import concourse.bass as bass
import concourse.mybir as mybir
import concourse.tile as tile

F32 = mybir.dt.float32
F32R = mybir.dt.float32r
BF16 = mybir.dt.bfloat16
EPS = 1e-6


def build_attention(nc, B_local, N, C, H, reps=1):
    AF = mybir.ActivationFunctionType
    Dh = C // H             # 64
    assert Dh == 64
    KT = C // 128           # 8 contraction k-tiles
    NT = N // 128           # 8 token m-tiles
    TCH = 512               # token chunk (PSUM free-dim limit)
    NCH = N // TCH          # 2
    FQK = 2 * C // 128      # 16 q+k feature tiles
    VW = 512                # v-weight chunk width
    NVC = C // VW           # 2
    E = Dh + 1              # 65: Dh v-features + ones column (denominator)

    # ---- external I/O ----
    xT = nc.dram_tensor("xT", [B_local, C, N], BF16, kind="ExternalInput").ap()
    qk_wT = nc.dram_tensor("qk_wT", [FQK, 128, C], BF16,
                           kind="ExternalInput").ap()
    v_wT = nc.dram_tensor("v_wT", [C, C], BF16, kind="ExternalInput").ap()
    proj_wT = nc.dram_tensor("proj_wT", [KT, 128, C], BF16,
                             kind="ExternalInput").ap()
    proj_b = nc.dram_tensor("proj_b", [C], F32, kind="ExternalInput").ap()
    mask_q = nc.dram_tensor("mask_q", [C, H], BF16, kind="ExternalInput").ap()
    selq = nc.dram_tensor("selq", [H, C], BF16, kind="ExternalInput").ap()
    selk = nc.dram_tensor("selk", [H, C], BF16, kind="ExternalInput").ap()
    yT = nc.dram_tensor("yT", [B_local, C, N], F32, kind="ExternalOutput").ap()

    with tile.TileContext(nc) as tc, ExitStack() as ctx:
        singles = ctx.enter_context(tc.tile_pool(name="singles", bufs=1))
        wqkp = ctx.enter_context(tc.tile_pool(name="wqkp", bufs=1))
        vwp = ctx.enter_context(tc.tile_pool(name="vwp", bufs=2))
        pwp = ctx.enter_context(tc.tile_pool(name="pwp", bufs=2))
        xp = ctx.enter_context(tc.tile_pool(name="xp", bufs=2))
        qkp = ctx.enter_context(tc.tile_pool(name="qkp", bufs=24))
        vap = ctx.enter_context(tc.tile_pool(name="vap", bufs=8))
        ptp = ctx.enter_context(tc.tile_pool(name="ptp", bufs=2))
        atp = ctx.enter_context(tc.tile_pool(name="atp", bufs=9))
        stagep = ctx.enter_context(tc.tile_pool(name="stagep", bufs=2))
        aop = ctx.enter_context(tc.tile_pool(name="aop", bufs=8))
        ystp = ctx.enter_context(tc.tile_pool(name="ystp", bufs=2))
        statp = ctx.enter_context(tc.tile_pool(name="statp", bufs=2))
        sqp = ctx.enter_context(tc.tile_pool(name="sqp", bufs=2))

        # PSUM: stps = attention score tiles (2 banks x 2); mmps = all
        # phase1/projection matmuls (1 bank x 2); avps = AV accumulator.
        stps = ctx.enter_context(tc.tile_pool(name="stps", bufs=2,
                                              space="PSUM"))
        mmps = ctx.enter_context(tc.tile_pool(name="mmps", bufs=2,
                                              space="PSUM"))
        avps = ctx.enter_context(tc.tile_pool(name="avps", bufs=1,
                                              space="PSUM"))

        # ---- constants ----
        mask_sb = singles.tile([128, KT, H], BF16)
        nc.sync.dma_start(out=mask_sb,
                          in_=mask_q.rearrange("(k p) h -> p k h", p=128))
        selq_sb = singles.tile([H, C], BF16)
        nc.sync.dma_start(out=selq_sb, in_=selq)
        selk_sb = singles.tile([H, C], BF16)
        nc.sync.dma_start(out=selk_sb, in_=selk)
        densel_sb = selk_sb
        bias_sb = singles.tile([128, KT], F32)
        nc.sync.dma_start(out=bias_sb, in_=proj_b.rearrange("(k p) -> p k",
                                                            p=128))
        eps_sb = singles.tile([H, 1], F32)
        nc.vector.memset(eps_sb, EPS)

        loop = ctx.enter_context(tc.For_i(0, reps, 1)) if reps > 1 else None

        state = {}

        x_pre = {}
        for tcn in range(NCH):
            xt0 = xp.tile([128, KT, TCH], BF16, tag="x", name=f"x0_{tcn}")
            nc.gpsimd.dma_start(
                out=xt0,
                in_=xT[0, :, tcn * TCH:(tcn + 1) * TCH].rearrange(
                    "(k p) t -> p k t", p=128))
            x_pre[tcn] = xt0

        wqk = []
        for ft in range(FQK):
            wt_r = wqkp.tile([128, KT, 128], BF16, name=f"wqk{ft}")
            nc.sync.dma_start(
                out=wt_r, in_=qk_wT[ft].rearrange("p (k f) -> p k f", f=128))
            wqk.append(wt_r)

        def emit_phase1(b, fast=False, x_pre=None):
            # during the lead-in (b==0) the attention stps pool is idle;
            # borrowing its 2 slots doubles phase1's matmul buffering, and
            # the avps slot holds a PSUM-accumulated ssq (instead of DVE
            # adds)
            mpool, mtag = (stps, "st") if fast else (mmps, "mm")
            # ssq_all layout: [H, g*N + tcn*TCH] for g in {q,k}, tcn halves
            ssq_all = statp.tile([H, 2 * N], F32, tag="ssqa", bufs=1,
                                 name=f"ssqa{b}")
            qk = {}
            va = {}
            x_sb = {}
            pend = None          # (sq tile, ssq_all column offset, is_first)
            seen = set()
            chain = {"ps": None, "cnt": 0}

            def flush_pend():
                nonlocal pend
                if pend is None:
                    return
                sq, off, first, fi = pend
                if fast:
                    if chain["cnt"] == 0:
                        chain["ps"] = avps.tile([H, TCH], F32, tag="av",
                                                name=f"ssqp{b}_{off}")
                    nc.tensor.matmul(chain["ps"], mask_sb[:, fi], sq,
                                     start=(chain["cnt"] == 0),
                                     stop=(chain["cnt"] == KT - 1))
                    chain["cnt"] += 1
                    if chain["cnt"] == KT:
                        nc.vector.tensor_copy(ssq_all[:, off:off + TCH],
                                              chain["ps"])
                        chain["ps"] = None
                        chain["cnt"] = 0
                else:
                    ps2 = mmps.tile([128, TCH], F32, tag="mm")
                    nc.tensor.matmul(ps2[:H], mask_sb[:, fi], sq,
                                     start=True, stop=True)
                    dst = ssq_all[:, off:off + TCH]
                    if first:
                        nc.vector.tensor_copy(dst, ps2[:H])
                    else:
                        nc.vector.tensor_add(dst, dst, ps2[:H])
                pend = None

            for tcn in range(NCH):
                if x_pre is not None:
                    xt = x_pre[tcn]
                else:
                    xt = xp.tile([128, KT, TCH], BF16, tag="x",
                                 name=f"x{b}_{tcn}")
                    nc.gpsimd.dma_start(
                        out=xt,
                        in_=xT[b, :, tcn * TCH:(tcn + 1) * TCH].rearrange(
                            "(k p) t -> p k t", p=128))
                x_sb[tcn] = xt
                tsl = slice(tcn * TCH, (tcn + 1) * TCH)
                for g in range(2):          # 0 = q tiles, 1 = k tiles
                    off = g * N + tcn * TCH
                    for fi in range(KT):
                        ft = g * KT + fi
                        if tcn == 0:
                            qk[ft] = qkp.tile([128, N], BF16, tag="qk",
                                              name=f"qk_{b}_{ft}")
                        ps = mpool.tile([128, TCH], F32, tag=mtag)
                        for k in range(KT):
                            nc.tensor.matmul(ps, wqk[ft][:, k], xt[:, k],
                                             start=(k == 0),
                                             stop=(k == KT - 1))
                        nc.vector.tensor_copy(qk[ft][:, tsl], ps)
                        sq = sqp.tile([128, TCH], BF16, tag="sq")
                        nc.vector.tensor_mul(sq, qk[ft][:, tsl],
                                             qk[ft][:, tsl])
                        flush_pend()
                        pend = (sq, off, off not in seen, fi)
                        seen.add(off)
            flush_pend()

            # one deferred rsqrt chain for all of q/k, both halves:
            # invr_all = 1/sqrt(ssq_all/Dh + eps)  (single ACT Sqrt op)
            nc.scalar.activation(out=ssq_all, in_=ssq_all, func=AF.Sqrt,
                                 bias=eps_sb, scale=1.0 / Dh)
            rtmp = statp.tile([H, 2 * N], F32, tag="scr", bufs=1,
                              name=f"rtmp{b}")
            nc.vector.reciprocal_approx_fast(out=rtmp, in_=ssq_all)
            invr_all = statp.tile([H, 2 * N], BF16, tag="invr", bufs=1,
                                  name=f"invr{b}")
            nc.vector.tensor_copy(invr_all, rtmp)

            # V-part interleaved with the q/k scale pass (keeps PE fed while
            # DVE does the scale muls)
            scale_steps = [(ft, half) for ft in range(FQK)
                           for half in range(NCH)]
            si = 0

            def emit_scale(n):
                nonlocal si
                for _ in range(n):
                    if si >= len(scale_steps):
                        return
                    ft, half = scale_steps[si]
                    si += 1
                    g, fi = divmod(ft, KT)
                    sel = selq_sb if g == 0 else selk_sb
                    iv = invr_all[:, g * N + half * TCH:
                                  g * N + (half + 1) * TCH]
                    bs = mmps.tile([128, TCH], F32, tag="mm")
                    nc.tensor.matmul(bs, sel[:, fi * 128:(fi + 1) * 128],
                                     iv, start=True, stop=True)
                    hs = slice(half * TCH, (half + 1) * TCH)
                    nc.vector.tensor_mul(qk[ft][:, hs], qk[ft][:, hs], bs)

            for vc in range(NVC):
                vwt = vwp.tile([128, KT, VW], BF16, tag="vw")
                nc.gpsimd.dma_start(
                    out=vwt,
                    in_=v_wT[:, vc * VW:(vc + 1) * VW].rearrange(
                        "(k p) f -> p k f", p=128))
                for j in range(NT):
                    if vc == 0:
                        va[j] = vap.tile([128, H, E], BF16, tag="va",
                                         name=f"va_{b}_{j}")
                        nc.vector.memset(va[j][:, :, Dh:E], 1.0)
                    ps = mpool.tile([128, TCH], F32, tag=mtag)
                    xsrc = x_sb[j // (TCH // 128)]
                    tm = j % (TCH // 128)
                    for k in range(KT):
                        nc.tensor.matmul(
                            ps[:, :VW], xsrc[:, k, tm * 128:(tm + 1) * 128],
                            vwt[:, k], start=(k == 0), stop=(k == KT - 1))
                    nc.vector.tensor_copy(
                        va[j][:, vc * (VW // Dh):(vc + 1) * (VW // Dh), 0:Dh],
                        ps[:, :VW].rearrange("p (h e) -> p h e", e=Dh))
                    emit_scale(2)
            emit_scale(len(scale_steps))
            state[b] = dict(qk=qk, va=va)

        def emit_attn(b):
            qk = state[b]["qk"]
            va = state[b]["va"]
            den = statp.tile([H, N], BF16, tag="den", name=f"den{b}")
            ao_all = {}
            for f in range(KT):
                qt = qk[f]
                kt = qk[KT + f]
                ao = aop.tile([128, N], BF16, tag="ao", name=f"attn_{b}_{f}")
                for h2 in range(2):
                    h = 2 * f + h2
                    psl = slice(h2 * Dh, (h2 + 1) * Dh)
                    av = avps.tile([E, N], F32, tag="av")
                    for j in range(NT):
                        st = stps.tile([128, N], F32, tag="st")
                        for half in range(NCH):
                            hs = slice(half * TCH, (half + 1) * TCH)
                            nc.tensor.matmul(
                                st[:, hs],
                                kt[psl, j * 128:(j + 1) * 128],
                                qt[psl, hs], start=True, stop=True)
                        pt = ptp.tile([128, N], BF16, tag="pt")
                        nc.scalar.activation(out=pt, in_=st, func=AF.Exp)
                        for half in range(NCH):
                            hs = slice(half * TCH, (half + 1) * TCH)
                            nc.tensor.matmul(av[:, hs], va[j][:, h, :],
                                             pt[:, hs], start=(j == 0),
                                             stop=(j == NT - 1))
                    stg = stagep.tile([E, N], BF16, tag="stage")
                    nc.vector.tensor_copy(stg, av[0:E, :])
                    nc.sync.dma_start(out=ao[psl, :], in_=stg[0:Dh, :])
                    nc.sync.dma_start(out=den[h:h + 1, :], in_=stg[Dh:E, :])
                ao_all[f] = ao
            state[b]["ao"] = ao_all
            state[b]["den"] = den
            state[b]["qk"] = None
            state[b]["va"] = None

        def emit_proj(b, fast=False):
            mpool, mtag = (stps, "st") if fast else (mmps, "mm")
            ao = state[b]["ao"]
            den = state[b]["den"]
            # invden = 1/den (upcast bf16->f32, DVE approx recip, round bf16)
            dscr = statp.tile([H, 2 * N], F32, tag="scr", bufs=1,
                              name=f"dscr{b}")
            nc.vector.tensor_copy(dscr[:, 0:N], den)
            nc.vector.reciprocal_approx_fast(out=dscr[:, N:2 * N],
                                             in_=dscr[:, 0:N])
            invden = statp.tile([H, N], BF16, tag="invdr", bufs=1,
                                name=f"invd{b}")
            nc.vector.tensor_copy(invden, dscr[:, N:2 * N])

            for half in range(NCH):
                hs = slice(half * TCH, (half + 1) * TCH)
                at = []
                for f in range(KT):
                    bd = mmps.tile([128, TCH], F32, tag="mm")
                    nc.tensor.matmul(bd,
                                     densel_sb[:, f * 128:(f + 1) * 128],
                                     invden[:, hs], start=True, stop=True)
                    a = atp.tile([128, TCH], BF16, tag="at")
                    nc.vector.tensor_mul(a, ao[f][:, hs], bd)
                    at.append(a)
                for mt in range(KT):
                    pwt = pwp.tile([128, KT, 128], BF16, tag="pw")
                    nc.gpsimd.dma_start(
                        out=pwt,
                        in_=proj_wT[mt].rearrange("p (k f) -> p k f", f=128))
                    ps = mpool.tile([128, TCH], F32, tag=mtag)
                    for k in range(KT):
                        nc.tensor.matmul(ps, pwt[:, k], at[k],
                                         start=(k == 0), stop=(k == KT - 1))
                    yst = ystp.tile([128, TCH], F32, tag="yst")
                    nc.vector.tensor_scalar_add(yst, ps,
                                                bias_sb[:, mt:mt + 1])
                    nc.sync.dma_start(out=yT[b, mt * 128:(mt + 1) * 128, hs],
                                      in_=yst)
                del at

        # emission order interleaves batches so phase1(b+1) can overlap
        # attention(b) on the free PSUM/PE slots, and proj(b) overlaps
        # attention(b+1).
        emit_phase1(0, fast=True, x_pre=x_pre)
        emit_attn(0)
        for b in range(1, B_local):
            emit_phase1(b)
            emit_proj(b - 1)
            emit_attn(b)
        emit_proj(B_local - 1, fast=True)

    return nc


def prep_inputs(x, qkv_w, proj_w, proj_b, q_norm_w, k_norm_w, n_cores):
    """Host-side prep: shard over batch, pre-transpose, cast to bf16, build
    selector masks.  Returns (in_maps, meta)."""
    import ml_dtypes

    bf16 = ml_dtypes.bfloat16
    B, N, C = x.shape
    H = C // 64
    Dh = 64
    B_local = B // n_cores
    scale = Dh ** -0.5

    qkv_wT = np.ascontiguousarray(qkv_w.T)          # [C, 3C]
    qk_wT = np.ascontiguousarray(
        qkv_wT[:, :2 * C].reshape(C // 128, 128, 2 * C // 128, 128)
        .transpose(2, 1, 0, 3).reshape(2 * C // 128, 128, C)).astype(bf16)
    v_wT = np.ascontiguousarray(qkv_wT[:, 2 * C:]).astype(bf16)
    proj_wT = np.ascontiguousarray(
        proj_w.T.reshape(C // 128, 128, C // 128, 128)
        .transpose(2, 1, 0, 3).reshape(C // 128, 128, C)).astype(bf16)

    heads = np.arange(C) // Dh
    mask_q = (heads[:, None] == np.arange(H)[None, :]).astype(bf16)
    w_qk = (q_norm_w * k_norm_w).astype(np.float32)  # [Dh]
    selq = np.zeros((H, C), np.float32)
    selk = np.zeros((H, C), np.float32)
    for h in range(H):
        selq[h, h * Dh:(h + 1) * Dh] = scale * w_qk
        selk[h, h * Dh:(h + 1) * Dh] = 1.0

    shared = dict(qk_wT=qk_wT, v_wT=v_wT, proj_wT=proj_wT,
                  proj_b=np.asarray(proj_b, np.float32), mask_q=mask_q,
                  selq=selq.astype(bf16), selk=selk.astype(bf16))
    in_maps = []
    for i in range(n_cores):
        xs = x[i * B_local:(i + 1) * B_local]        # [B_local, N, C]
        xTl = np.ascontiguousarray(xs.transpose(0, 2, 1)).astype(bf16)
        in_maps.append(dict(xT=xTl, **shared))
    return in_maps, dict(B=B, N=N, C=C, H=H, B_local=B_local)


def gather_output(results, meta):
    B, N, C, B_local = meta["B"], meta["N"], meta["C"], meta["B_local"]
    y = np.empty((B, N, C), np.float32)
    for i, r in enumerate(results):
        yTl = r["yT"]                                # [B_local, C, N]
        y[i * B_local:(i + 1) * B_local] = yTl.transpose(0, 2, 1)
    return y


N_CORES = 8
_CACHE = {}


def _get_nc():
    if "nc" not in _CACHE:
        from concourse import bacc

        nc = bacc.Bacc("TRN2", target_bir_lowering=False, debug=False,
                       num_devices=N_CORES)
        build_attention(nc, B_local=16 // N_CORES, N=1024, C=1024, H=16)
        nc.compile()
        _CACHE["nc"] = nc
    return _CACHE["nc"]


def run_sharded(in_maps, trace=False):
    from concourse.bass_utils import run_bass_kernel_spmd

    return run_bass_kernel_spmd(_get_nc(), in_maps,
                                core_ids=list(range(N_CORES)), trace=trace)


def kernel(x, qkv_w, proj_w, proj_b, q_norm_w, k_norm_w):
    in_maps, meta = prep_inputs(np.asarray(x), np.asarray(qkv_w),
                                np.asarray(proj_w), np.asarray(proj_b),
                                np.asarray(q_norm_w), np.asarray(k_norm_w),
                                N_CORES)
    res = run_sharded(in_maps)
    return gather_output(res.results, meta)

